# revision 16
# baseline (speedup 1.0000x reference)
"""Causal self-attention (RoPE, 16 heads) Trainium2 Bass kernel.

Problem: B=8, S=1024, D=1024, H=16, HS=64, fp32 in/out, causal mask,
all-ones padding mask.

Strategy: data-parallel over batch — one batch element per NeuronCore.
All matmuls in bf16 (inputs cast on host; PSUM accumulation stays fp32),
which keeps the PE at 1 cycle/row at any tile size and halves DVE /
LDWEIGHTS / DMA cost. Host supplies x^T directly so there is no on-chip
transpose phase.

Per-core flow (everything in a "transposed activation" layout):
  x^T   [D, S]  bf16, DMA'd straight in (host transposes, free)
  Q^T,K^T [D,S] = W^T @ x^T with RoPE fused on eviction. The host permutes
                  W_q/W_k columns so each rope pair (x1,x2) sits 16 rows
                  apart inside a 32-partition quadrant; the rope "swap" is
                  then a single DVE stream_shuffle (mask = halves of each
                  quadrant) instead of 4 partition-copies:
                    dst = e*c1 + shuffle16(e)*c2   (c1/c2 host tiles, bf16;
                  Q side pre-scaled by 1/sqrt(hs))
  V     [S, D]  = x @ W_v (lhsT = x^T chunks), stored per-head with an
                  appended ones-column so att@v also yields softmax sums
  S^T   [k, q]  per head, 128x512 causal blocks only, trimmed to the live
                  causal span; both heads of a 128-row tile share one
                  2-bank PSUM tile [128, 2, 512] so exp runs as ONE ACT
                  instruction over both heads
  att^T         = exp(S^T) (scores are small — no max subtraction), written
                  into persistent bf16 tiles whose causally-dead columns are
                  zeroed once at startup; diagonal blocks masked by a 0/1
                  triangle multiply
  y^T   [D, S]  accumulated per head: lhsT = [v | 1] chunk, rhs = att^T;
                  row 64 gives softmax sums; normalize via reciprocal of
                  that row + gpsimd partition_broadcast, fused into the
                  PSUM->SBUF eviction multiply
  out   [S, D]  = y @ W_proj (lhsT = y^T chunks), fp32 out

PE emission order is software-pipelined: scores(ft) -> QK(ft+1) ->
att@v(ft), so exp(ft) on ACT hides under the next head-pair's projection
matmuls and the PE never idles (stays at the 2.4 GHz p-state).
"""

import os

# The Bass kernel executes through the axon PJRT backend and needs the
# NeuronCores visible; a JAX_PLATFORMS=cpu pin (used for jax reference
# computation) would hide them.
if "axon" not in os.environ.get("JAX_PLATFORMS", "axon"):
    os.environ.pop("JAX_PLATFORMS", None)

import numpy as np
import ml_dtypes
from contextlib import ExitStack

import concourse.bass as bass
import concourse.mybir as mybir
import concourse.tile as tile
from concourse import bacc
from concourse.bass_utils import run_bass_kernel_spmd

B, S, D, H, HS = 8, 1024, 1024, 16, 64
P = 128
NCORES = 8
F32 = mybir.dt.float32
BF16 = mybir.dt.bfloat16
EXP = mybir.ActivationFunctionType.Exp
BFNP = ml_dtypes.bfloat16

# swap the two 16-row halves of every 32-partition quadrant
SHUF16 = list(range(16, 32)) + list(range(0, 16))

_CACHE = {}
DBG = os.environ.get("BASS_DBG", "0") == "1"


def _build_nc():
    nc = bacc.Bacc(
        "TRN2", target_bir_lowering=False, debug=False, num_devices=NCORES)
    xt_d = nc.dram_tensor("xt", [D, S], BF16, kind="ExternalInput")
    wq_d = nc.dram_tensor("wq", [D, D], BF16, kind="ExternalInput")
    wk_d = nc.dram_tensor("wk", [D, D], BF16, kind="ExternalInput")
    wv_d = nc.dram_tensor("wv", [D, D], BF16, kind="ExternalInput")
    wp_d = nc.dram_tensor("wp", [D, D], BF16, kind="ExternalInput")
    c1q_d = nc.dram_tensor("c1q", [P, S], BF16, kind="ExternalInput")
    c2q_d = nc.dram_tensor("c2q", [P, S], BF16, kind="ExternalInput")
    c1k_d = nc.dram_tensor("c1k", [P, S], BF16, kind="ExternalInput")
    c2k_d = nc.dram_tensor("c2k", [P, S], BF16, kind="ExternalInput")
    mask2_d = nc.dram_tensor("mask2", [P, 2 * P], BF16, kind="ExternalInput")
    ones_d = nc.dram_tensor("ones", [P, H], BF16, kind="ExternalInput")
    out_d = nc.dram_tensor("out", [S, D], F32, kind="ExternalOutput")
    if DBG:
        dbgq_d = nc.dram_tensor("dbgq", [D, S], BF16, kind="ExternalOutput")
        dbgk_d = nc.dram_tensor("dbgk", [D, S], BF16, kind="ExternalOutput")
        dbgv_d = nc.dram_tensor("dbgv", [S, H * (HS + 1)], BF16,
                                kind="ExternalOutput")
        dbgy_d = nc.dram_tensor("dbgy", [D, S], BF16, kind="ExternalOutput")
        dbga_d = nc.dram_tensor("dbga", [2 * P, 2 * 512], BF16,
                                kind="ExternalOutput")

    def mm(out, lhsT, rhs, start, stop):
        nc.tensor.matmul(out, lhsT, rhs, start=start, stop=stop)

    with tile.TileContext(nc) as tc, ExitStack() as ctx:
        persist = ctx.enter_context(tc.tile_pool(name="persist", bufs=1))
        xt = [persist.tile([P, S], BF16, name=f"xt{i}", tag=f"xt{i}")
              for i in range(8)]
        qt = [persist.tile([P, S], BF16, name=f"qt{i}", tag=f"qt{i}")
              for i in range(8)]
        kt = [persist.tile([P, S], BF16, name=f"kt{i}", tag=f"kt{i}")
              for i in range(8)]
        vt = [persist.tile([P, H, HS + 1], BF16, name=f"vt{i}", tag=f"vt{i}")
              for i in range(8)]
        yt = [persist.tile([P, S], BF16, name=f"yt{i}", tag=f"yt{i}")
              for i in range(8)]
        c1q = persist.tile([P, S], BF16, name="c1q_t", tag="c1q_t")
        c2q = persist.tile([P, S], BF16, name="c2q_t", tag="c2q_t")
        c1k = persist.tile([P, S], BF16, name="c1k_t", tag="c1k_t")
        c2k = persist.tile([P, S], BF16, name="c2k_t", tag="c2k_t")
        mask2 = persist.tile([P, 2 * P], BF16, name="mask2_t", tag="mask2_t")
        ones_t = persist.tile([P, H], BF16, name="ones_t", tag="ones_t")
        for t, d_ in ((c1q, c1q_d), (c2q, c2q_d), (c1k, c1k_d), (c2k, c2k_d),
                      (mask2, mask2_d), (ones_t, ones_d)):
            nc.sync.dma_start(t[:], d_[:])
        mask_ap = mask2[:].rearrange("p (a b) -> p a b", b=P)
        for i in range(8):
            nc.sync.dma_start(xt[i][:], xt_d[i * P:(i + 1) * P, :])

        # persistent att tiles, double-buffered by ft parity; dead causal
        # columns zeroed once here and never written again
        KMAX = (4, 8)
        att_t = [[[persist.tile([P, 2, 512], BF16, name=f"att{pa}_{qc}_{kc}",
                                tag=f"att{pa}_{qc}_{kc}")
                   for kc in range(KMAX[qc])] for qc in range(2)]
                 for pa in range(2)]
        for pa in range(2):
            for qc in range(2):
                for kc in range(KMAX[qc]):
                    qsub = max(0, kc * P - qc * 512)
                    if qsub > 0:
                        nc.vector.memset(att_t[pa][qc][kc][:, :, 0:qsub], 0)

        wst = ctx.enter_context(tc.tile_pool(name="wst", bufs=34))
        wvp = ctx.enter_context(tc.tile_pool(name="wvp", bufs=9))
        rtp = ctx.enter_context(tc.tile_pool(name="rtp", bufs=3))
        rbp = ctx.enter_context(tc.tile_pool(name="rbp", bufs=3))
        outp = ctx.enter_context(tc.tile_pool(name="outp", bufs=4))
        pa_p = ctx.enter_context(tc.tile_pool(name="pa", bufs=2, space="PSUM"))
        pss_p = ctx.enter_context(tc.tile_pool(name="pss", bufs=2, space="PSUM"))
        psy_p = ctx.enter_context(tc.tile_pool(name="psy", bufs=2, space="PSUM"))

        def emit_qk(ft):
            wts = {}
            for key, wd in (("q", wq_d), ("k", wk_d)):
                wts[key] = []
                for dc in range(8):
                    wtile = wst.tile([P, P], BF16, name="wtile", tag="w")
                    nc.sync.dma_start(
                        wtile[:], wd[dc * P:(dc + 1) * P, ft * P:(ft + 1) * P])
                    wts[key].append(wtile)
            # s2=0 of both Q and K first: scores(ft, qc=0) only needs cols 0:512
            for s2 in range(2):
                for key, dst, c1, c2 in (("q", qt, c1q, c2q),
                                         ("k", kt, c1k, c2k)):
                    ps = pa_p.tile([P, 512], F32, name="qkps", tag="pa")
                    for dc in range(8):
                        mm(ps[:], wts[key][dc][:],
                           xt[dc][:, s2 * 512:(s2 + 1) * 512],
                           dc == 0, dc == 7)
                    # rope: dst = e*c1 + shuffle16(e)*c2
                    e = rtp.tile([P, 512], BF16, name="ropee", tag="e")
                    nc.scalar.copy(e[:], ps[:])
                    tsw = rtp.tile([P, 512], BF16, name="ropesw", tag="tsw")
                    nc.vector.stream_shuffle(tsw[:], e[:], SHUF16)
                    m1 = rtp.tile([P, 512], BF16, name="ropem1", tag="m1")
                    sl = slice(s2 * 512, (s2 + 1) * 512)
                    nc.vector.tensor_mul(m1[:], e[:], c1[:, sl])
                    nc.vector.tensor_mul(tsw[:], tsw[:], c2[:, sl])
                    nc.vector.tensor_add(dst[ft][:, sl], m1[:], tsw[:])

        def emit_scores(ft):
            pa = ft & 1
            for qc in range(2):
                for kc in range(KMAX[qc]):
                    qsub = max(0, kc * P - qc * 512)
                    pss = pss_p.tile([P, 2, 512], F32, name="pss", tag="pss")
                    for h in range(2):
                        mm(pss[:, h, qsub:512],
                           kt[ft][h * 64:h * 64 + 64, kc * P:(kc + 1) * P],
                           qt[ft][h * 64:h * 64 + 64,
                                  qc * 512 + qsub:(qc + 1) * 512],
                           True, True)
                    at = att_t[pa][qc][kc]
                    nc.scalar.activation(at[:, :, qsub:512],
                                         pss[:, :, qsub:512], EXP)
                    dm = kc * P - qc * 512
                    if 0 <= dm < 512:
                        with tc.high_priority(offset=150):
                            nc.vector.tensor_mul(at[:, :, dm:dm + P],
                                                 at[:, :, dm:dm + P], mask_ap)

        def emit_attv(ft):
            pa = ft & 1
            for qc in range(2):
                kmax = KMAX[qc]
                for h in range(2):
                    psy = psy_p.tile([P, 512], F32, name="psy", tag="psy")
                    for kc in range(kmax):
                        qsub = max(0, kc * P - qc * 512)
                        mm(psy[0:HS + 1, qsub:512], vt[kc][:, 2 * ft + h, :],
                           att_t[pa][qc][kc][:, h, qsub:512],
                           kc == 0, kc == kmax - 1)
                    hb = h * 64
                    sl = yt[ft][hb:hb + 64, qc * 512:(qc + 1) * 512]
                    with tc.high_priority(offset=200):
                        # free the psy bank ASAP: evict unnormalized + grab
                        # the sums row; normalization happens in-place on yt
                        # off the PSUM critical path
                        nc.vector.tensor_copy(sl, psy[0:HS, :])
                        srow = rbp.tile([1, 512], F32, name="srow", tag="srow")
                        nc.vector.tensor_copy(srow[:], psy[HS:HS + 1, :])
                    rrow = rbp.tile([1, 512], F32, name="rrow", tag="rrow")
                    nc.vector.reciprocal_approx_fast(out=rrow[:], in_=srow[:])
                    rb = rbp.tile([P, 512], F32, name="rb", tag="rb")
                    nc.gpsimd.partition_broadcast(rb[:], rrow[0:1, :])
                    nc.vector.tensor_mul(sl, sl, rb[hb:hb + 64, :])

        # ---------------- emission ----------------
        emit_qk(0)

        # V phase: V = x @ W_v, stored per-head + ones column
        for f2 in range(2):
            wvts = []
            for dc in range(8):
                wvtile = wvp.tile([P, 512], BF16, name="wvtile", tag="wv")
                nc.sync.dma_start(
                    wvtile[:], wv_d[dc * P:(dc + 1) * P, f2 * 512:(f2 + 1) * 512])
                wvts.append(wvtile)
            for sc in range(8):
                ps = pa_p.tile([P, 512], F32, name="vps", tag="pa")
                for dc in range(8):
                    mm(ps[:], xt[dc][:, sc * P:(sc + 1) * P], wvts[dc][:],
                       dc == 0, dc == 7)
                if f2 == 0:
                    nc.vector.tensor_copy(
                        vt[sc][:, 0:8, 0:HS],
                        ps[:].rearrange("p (h e) -> p h e", e=HS))
                else:
                    nc.scalar.copy(
                        vt[sc][:, 8:16, 0:HS],
                        ps[:].rearrange("p (h e) -> p h e", e=HS))
        for sc in range(8):
            nc.vector.tensor_copy(vt[sc][:, :, HS], ones_t[:])

        # prefetch first half of W_proj during the attention phase
        wp_pref = []
        for dc in range(8):
            wptile = wvp.tile([P, 512], BF16, name="wptile", tag="wv")
            nc.sync.dma_start(wptile[:], wp_d[dc * P:(dc + 1) * P, 0:512])
            wp_pref.append(wptile)

        for ft in range(8):
            emit_scores(ft)
            if ft < 7:
                emit_qk(ft + 1)
            emit_attv(ft)

        if DBG:
            for i in range(8):
                nc.sync.dma_start(dbgq_d[i * P:(i + 1) * P, :], qt[i][:])
                nc.sync.dma_start(dbgk_d[i * P:(i + 1) * P, :], kt[i][:])
                nc.sync.dma_start(dbgy_d[i * P:(i + 1) * P, :], yt[i][:])
                nc.sync.dma_start(
                    dbgv_d[i * P:(i + 1) * P, :],
                    vt[i][:].rearrange("p h e -> p (h e)"))
            # att tiles for ft=7 (parity 1): qc=1, kc=0 and kc=7
            nc.sync.dma_start(dbga_d[0:P, :],
                              att_t[1][1][0][:].rearrange("p a b -> p (a b)"))
            nc.sync.dma_start(dbga_d[P:2 * P, :],
                              att_t[1][1][7][:].rearrange("p a b -> p (a b)"))

        # output projection
        for n2 in range(2):
            if n2 == 0:
                wpts = wp_pref
            else:
                wpts = []
                for dc in range(8):
                    wptile = wvp.tile([P, 512], BF16, name="wptile", tag="wv")
                    nc.sync.dma_start(
                        wptile[:],
                        wp_d[dc * P:(dc + 1) * P, n2 * 512:(n2 + 1) * 512])
                    wpts.append(wptile)
            for sc in range(8):
                psp = pa_p.tile([P, 512], F32, name="psp", tag="pa")
                for dc in range(8):
                    mm(psp[:], yt[dc][:, sc * P:(sc + 1) * P], wpts[dc][:],
                       dc == 0, dc == 7)
                ot = outp.tile([P, 512], F32, name="ot", tag="ot")
                nc.scalar.copy(ot[:], psp[:])
                nc.sync.dma_start(
                    out_d[sc * P:(sc + 1) * P, n2 * 512:(n2 + 1) * 512], ot[:])
    nc.compile()
    return nc


def _prep(inputs):
    w_qkv = np.asarray(inputs["w_qkv"], np.float32)
    w_proj = np.asarray(inputs["w_proj"], np.float32)
    cos = np.asarray(inputs["cos"], np.float32).reshape(S, HS // 2)
    sin = np.asarray(inputs["sin"], np.float32).reshape(S, HS // 2)
    wq, wk, wv = w_qkv[:, 0:D], w_qkv[:, D:2 * D], w_qkv[:, 2 * D:3 * D]
    # quadrant-local rope pairing: within each head's 64 columns, quadrant
    # q2 holds pairs 16*q2..16*q2+15 as (x1 rows 0-15, x2 rows 16-31)
    perm = np.empty(D, np.int64)
    for h in range(H):
        b0 = h * HS
        for q2 in range(2):
            base = b0 + 32 * q2
            pr = 2 * (16 * q2 + np.arange(16))
            perm[base:base + 16] = b0 + pr
            perm[base + 16:base + 32] = b0 + pr + 1
    wq, wk = wq[:, perm], wk[:, perm]
    cosT = np.ascontiguousarray(cos.T)  # [32, S], row = freq index
    sinT = np.ascontiguousarray(sin.T)
    c1 = np.concatenate([cosT[0:16], cosT[0:16], cosT[16:32], cosT[16:32]] * 2, 0)
    c2 = np.concatenate([-sinT[0:16], sinT[0:16], -sinT[16:32], sinT[16:32]] * 2, 0)
    scale = np.float32(1.0 / np.sqrt(HS))
    mask = np.triu(np.ones((P, P), np.float32))  # [k, q]: allow q >= k
    mask2 = np.concatenate([mask, mask], 1)

    def bf(a):
        return np.ascontiguousarray(a).astype(BFNP)

    common = {
        "wq": bf(wq), "wk": bf(wk), "wv": bf(wv), "wp": bf(w_proj),
        "c1q": bf(c1 * scale), "c2q": bf(c2 * scale),
        "c1k": bf(c1), "c2k": bf(c2),
        "mask2": bf(mask2), "ones": bf(np.ones((P, H), np.float32)),
    }
    return common


LAST_RESULT = None


def kernel(**inputs):
    global LAST_RESULT
    if "nc" not in _CACHE:
        _CACHE["nc"] = _build_nc()
    nc = _CACHE["nc"]
    common = _prep(inputs)
    x = np.asarray(inputs["x"], np.float32)
    in_maps = [dict(common, xt=np.ascontiguousarray(x[b].T).astype(BFNP))
               for b in range(B)]
    res = run_bass_kernel_spmd(nc, in_maps, list(range(NCORES)))
    LAST_RESULT = res
    out = np.stack([res.results[i]["out"] for i in range(B)], 0)
    return out.astype(np.float32)
